# revision 11
# baseline (speedup 1.0000x reference)
"""Trainium2 Bass kernel for nn_Attention_40407052320989.

Causal GQA attention block (Llama-style): QKV projection + RoPE + causal
softmax attention (8 KV heads, 32 Q heads, n_rep=4) + output projection.

Sharding: tensor-parallel over heads across 8 NeuronCores. Core c owns
KV head c and its 4 query heads: Wq/Wk/Wv column-sharded, Wo row-sharded
by the same head group. Each core computes a full [B, S, D] partial of
the output (its head group's contribution through Wo); the host sums the
8 partials (the row-parallel unshard).

On-chip layout: "feature on partitions, tokens on free dim" everywhere.
Scores are computed transposed (scoresT[t, s]) so the exp'd tile feeds
the PV matmul directly as the moving operand with no transposes. Softmax
sums over t (partition dim) ride a ones-vector matmul; the normalizer is
broadcast back over partitions with a rank-1 matmul and inverted as a
full-width DVE reciprocal.

Phase overlap: attention (phase 2) is Scalar-engine-bound (the exp
chain) while projections (phase 1) and the output projection (phase 3)
are TensorE-bound. Emission order interleaves P2(b0) with P1(b1) and
P2(b1) with P3(b0) so the scheduler overlaps them; PSUM is partitioned
4 banks for projections (two-pass: Q then KV, x re-streamed) and 4 for
attention, with phase 3 taking over projection banks afterwards.

RoPE pairing: Wq/Wk columns are permuted host-side so rotation pairs
(2i, 2i+1) land at partitions (i, 64+i) (rotate-half layout). Scores
are invariant to a consistent head-dim permutation of Wq and Wk. The
rotation is out = q * cos2 + swap_halves(q) * sin2n with cos2 = [cos;
cos] and sin2n = [-sin; sin]; the halves swap is two SBUF->SBUF DMAs.

Matmul operands are bf16 (measured: f32r runs ~2 cyc/row on HW, bf16 1
cyc/row); PSUM accumulation, softmax normalization, and the output stay
fp32. End-to-end numpy simulation of this precision mix: 4e-3 max rel.
"""

import math
import sys

sys.path.insert(0, "/opt/trn_rl_repo")

import ml_dtypes
import numpy as np

import concourse.bass as bass
import concourse.mybir as mybir
import concourse.tile as tile
from concourse import bacc
from concourse.bass_utils import run_bass_kernel_spmd

F32 = mybir.dt.float32
F32R = mybir.dt.float32r
BF16 = mybir.dt.bfloat16
AF = mybir.ActivationFunctionType
NP_BF16 = ml_dtypes.bfloat16

BSZ, SEQLEN, DIM = 2, 2048, 4096
N_HEADS, N_KV_HEADS, HEAD_DIM = 32, 8, 128
N_REP = N_HEADS // N_KV_HEADS  # q heads per core
N_CORES = 8
P = 128
NKD = DIM // P          # 32 contraction chunks for the projections
NT512 = SEQLEN // 512   # 4 blocks of 512 tokens per batch
NTC = SEQLEN // P       # 16 chunks of 128 tokens per batch
SCALE = 1.0 / math.sqrt(HEAD_DIM)

_CACHED = {}


def ts(i, n):
    return slice(i * n, (i + 1) * n)


class _Ctx:
    """Shared tiles/pools threaded through the phase emitters."""


def _build_nc():
    nc = bacc.Bacc(None, target_bir_lowering=False, debug=False)

    c = _Ctx()
    c.nc = nc
    c.xT = nc.declare_dram_parameter("xT", [BSZ, DIM, SEQLEN], BF16, isOutput=False)
    c.wq = nc.declare_dram_parameter("wq", [DIM, N_REP * HEAD_DIM], BF16, isOutput=False)
    c.wkv = nc.declare_dram_parameter("wkv", [DIM, 2 * HEAD_DIM], BF16, isOutput=False)
    c.wo = nc.declare_dram_parameter("wo", [N_REP * HEAD_DIM, DIM], BF16, isOutput=False)
    cos2 = nc.declare_dram_parameter("cos2", [P, SEQLEN], F32, isOutput=False)
    sin2n = nc.declare_dram_parameter("sin2n", [P, SEQLEN], F32, isOutput=False)
    dmask = nc.declare_dram_parameter("dmask", [P, 4, 512], BF16, isOutput=False)
    ident = nc.declare_dram_parameter("ident", [P, P], BF16, isOutput=False)
    ones_c = nc.declare_dram_parameter("ones_c", [P, 1], BF16, isOutput=False)
    ones_r = nc.declare_dram_parameter("ones_r", [1, P], F32R, isOutput=False)
    c.y = nc.declare_dram_parameter("y", [BSZ, SEQLEN, DIM], F32, isOutput=True)

    with tile.TileContext(nc) as tc, nc.allow_low_precision(
        reason="psum accumulation and normalization stay fp32 by construction"
    ):
        c.tc = tc
        with tc.tile_pool(name="const", bufs=1) as cpool:
            c.cos_sb = cpool.tile([P, SEQLEN], F32)
            c.sin_sb = cpool.tile([P, SEQLEN], F32)
            c.dm_sb = cpool.tile([P, 4, 512], BF16)
            c.id_sb = cpool.tile([P, P], BF16)
            c.onec_sb = cpool.tile([P, 1], BF16)
            c.oner_sb = cpool.tile([1, P], F32R)
            nc.sync.dma_start(c.cos_sb[:], cos2[:])
            nc.sync.dma_start(c.sin_sb[:], sin2n[:])
            nc.sync.dma_start(c.dm_sb[:], dmask[:])
            nc.sync.dma_start(c.id_sb[:], ident[:])
            nc.sync.dma_start(c.onec_sb[:], ones_c[:])
            nc.sync.dma_start(c.oner_sb[:], ones_r[:])
            _emit(c)

    nc.compile()
    return nc


def _emit(c):
    nc, tc = c.nc, c.tc
    with tc.tile_pool(name="big", bufs=1) as big, \
         tc.tile_pool(name="xs", bufs=6) as xpool, \
         tc.tile_pool(name="tp", bufs=2) as tpool, \
         tc.tile_pool(name="ep", bufs=6) as epool, \
         tc.tile_pool(name="np_", bufs=2) as npool:
        c.xpool, c.tpool, c.epool, c.npool = xpool, tpool, epool, npool

        c.wq_sb = big.tile([P, NKD, N_REP * HEAD_DIM], BF16)
        c.wkv_sb = big.tile([P, NKD, 2 * HEAD_DIM], BF16)
        c.wo_sb = big.tile([P, N_REP, DIM], BF16)
        wq_r = c.wq.rearrange("(o p) m -> p o m", p=P)
        wkv_r = c.wkv.rearrange("(o p) m -> p o m", p=P)
        wo_r = c.wo.rearrange("(o p) n -> p o n", p=P)
        for o4 in range(4):
            nc.sync.dma_start(c.wq_sb[:, ts(o4, 8), :], wq_r[:, ts(o4, 8), :])
            nc.sync.dma_start(c.wkv_sb[:, ts(o4, 8), :], wkv_r[:, ts(o4, 8), :])
            nc.sync.dma_start(c.wo_sb[:, o4, :], wo_r[:, o4, :])

        for b in range(2):
            with tc.tile_pool(name=f"qkv{b}", bufs=1) as qkv:
                c.qt = {b: qkv.tile([P, N_REP, SEQLEN], BF16, name=f"qt{b}")}
                c.kt = {b: qkv.tile([P, SEQLEN], BF16, name=f"kt{b}")}
                c.vn = {b: qkv.tile([P, NTC, HEAD_DIM], BF16, name=f"vn{b}")}
                c.ao = {b: qkv.tile([P, N_REP, SEQLEN], BF16, name=f"ao{b}")}
                with tc.tile_pool(name=f"p1_{b}", bufs=1, space="PSUM") as p1:
                    c.p1 = p1
                    for t5 in range(NT512):
                        _p1_block(c, b, t5)
                with tc.tile_pool(name=f"p2_{b}", bufs=1, space="PSUM") as p2:
                    c.p2 = p2
                    for h in range(N_REP):
                        _p2_head(c, b, h)
                with tc.tile_pool(name=f"p3_{b}", bufs=1, space="PSUM") as p3, \
                     tc.tile_pool(name=f"op{b}", bufs=3) as opool:
                    c.p3, c.opool = p3, opool
                    for s1 in range(NTC):
                        _p3_row(c, b, s1)


def _rope(c, out_slice, psum_in, tsl):
    """out = psum_in * cos2 + swap_halves(psum_in) * sin2n, [128, 512]."""
    nc = c.nc
    qf = c.tpool.tile([P, 512], F32, tag="rope_qf")
    rot = c.tpool.tile([P, 512], F32, tag="rope_rot")
    tmpa = c.tpool.tile([P, 512], F32, tag="rope_tmpa")
    nc.any.tensor_copy(qf[:], psum_in[:])
    nc.sync.dma_start(rot[0:64, :], qf[64:128, :])
    nc.sync.dma_start(rot[64:128, :], qf[0:64, :])
    nc.vector.tensor_mul(tmpa[:], qf[:], c.cos_sb[:, tsl])
    nc.vector.tensor_mul(rot[:], rot[:], c.sin_sb[:, tsl])
    nc.vector.tensor_add(out_slice, tmpa[:], rot[:])


def _p1_block(c, b, t5):
    """Projections for one 512-token block (single pass, 6 accumulators)."""
    nc = c.nc
    tsl = ts(t5, 512)
    pq = [c.p1.tile([P, 512], F32, tag=f"pq{h}", name=f"pq{h}")
          for h in range(N_REP)]
    pk = c.p1.tile([P, 512], F32, tag="pk")
    pv = c.p1.tile([P, 512], F32, tag="pv")
    for kd in range(NKD):
        xt = c.xpool.tile([P, 512], BF16, tag="xt")
        nc.sync.dma_start(xt[:], c.xT[b, ts(kd, P), tsl])
        st, sp = kd == 0, kd == NKD - 1
        for h in range(N_REP):
            nc.tensor.matmul(pq[h][:], c.wq_sb[:, kd, ts(h, P)], xt[:],
                             start=st, stop=sp)
        nc.tensor.matmul(pk[:], c.wkv_sb[:, kd, 0:P], xt[:], start=st, stop=sp)
        nc.tensor.matmul(pv[:], c.wkv_sb[:, kd, P:2 * P], xt[:], start=st, stop=sp)
    _rope(c, c.kt[b][:, tsl], pk, tsl)
    for h in range(N_REP):
        _rope(c, c.qt[b][:, h, tsl], pq[h], tsl)
    # V^T [d, t] -> V natural [t, d] via PE transpose
    vt_tmp = c.tpool.tile([P, 512], BF16, tag="vt_tmp")
    nc.any.tensor_copy(vt_tmp[:], pv[:])
    for j in range(4):
        pvt = c.p1.tile([P, P], BF16, tag="pvt", bufs=2, name="pvt")
        nc.tensor.transpose(pvt[:], vt_tmp[:, ts(j, P)], c.id_sb[:])
        nc.any.tensor_copy(c.vn[b][:, t5 * 4 + j, :], pvt[:])


def _p2_head(c, b, h):
    """Causal attention for one query head, scores transposed [t, s]."""
    nc = c.nc
    for s5 in range(NT512):
        po = c.p2.tile([P, 512], F32, tag="po", bufs=2)
        pz = c.p2.tile([1, 512], F32, tag="pz", bufs=2)
        ssl = ts(s5, 512)
        ntc = 4 * s5 + 4
        for tci in range(ntc):
            pscr = c.p2.tile([P, 512], F32, tag="ps", bufs=3, name="pscr")
            nc.tensor.matmul(pscr[:], c.kt[b][:, ts(tci, P)], c.qt[b][:, h, ssl],
                             start=True, stop=True)
            ex = c.epool.tile([P, 512], BF16, tag="ex")
            nc.scalar.activation(ex[:], pscr[:], AF.Exp, scale=SCALE)
            if tci >= 4 * s5:
                nc.vector.tensor_mul(ex[:], ex[:], c.dm_sb[:, tci - 4 * s5, :])
            st, sp = tci == 0, tci == ntc - 1
            nc.tensor.matmul(po[:], c.vn[b][:, tci, :], ex[:], start=st, stop=sp)
            nc.tensor.matmul(pz[:], c.onec_sb[:], ex[:], start=st, stop=sp)
        # broadcast sums over partitions, then invert at full width
        zs = c.npool.tile([1, 512], F32R, tag="zs")
        nc.vector.tensor_copy(zs[:], pz[:])
        pb = c.p2.tile([P, 512], F32, tag="ps", bufs=3, name="pb")
        nc.tensor.matmul(pb[:], c.oner_sb[:], zs[:], start=True, stop=True)
        rb = c.npool.tile([P, 512], F32, tag="rb")
        nc.vector.reciprocal(rb[:], pb[:])
        nc.vector.tensor_mul(c.ao[b][:, h, ssl], po[:], rb[:])


def _p3_row(c, b, s1):
    """Output projection for one 128-token row: two [128, 1024] groups x2."""
    nc = c.nc
    for half in range(4):
        pf = c.p3.tile([P, 2, 512], F32, tag="pf", bufs=2)
        for nq in range(2):
            n5 = half * 2 + nq
            for dh in range(N_REP):
                nc.tensor.matmul(pf[:, nq, :], c.ao[b][:, dh, ts(s1, P)],
                                 c.wo_sb[:, dh, ts(n5, 512)],
                                 start=dh == 0, stop=dh == N_REP - 1)
        ot = c.opool.tile([P, 1024], F32, tag="ot")
        nc.vector.tensor_copy(ot[:], pf[:])
        nc.sync.dma_start(c.y[b, ts(s1, P), ts(half, 1024)], ot[:])


def _prep_inputs(x, freqs_cos, freqs_sin, Wq, Wk, Wv, Wo):
    x = np.ascontiguousarray(np.asarray(x, dtype=np.float32))
    Wq = np.asarray(Wq, dtype=np.float32)
    Wk = np.asarray(Wk, dtype=np.float32)
    Wv = np.asarray(Wv, dtype=np.float32)
    Wo = np.asarray(Wo, dtype=np.float32)
    fc = np.asarray(freqs_cos, dtype=np.float32)
    fs = np.asarray(freqs_sin, dtype=np.float32)

    xT = np.ascontiguousarray(x.transpose(0, 2, 1)).astype(NP_BF16)  # [B, D, S]

    # rotate-half column permutation within each head
    perm = np.concatenate([np.arange(0, HEAD_DIM, 2), np.arange(1, HEAD_DIM, 2)])

    cos2 = np.concatenate([fc.T, fc.T], axis=0)       # [128, S]
    sin2n = np.concatenate([-fs.T, fs.T], axis=0)     # [128, S]

    # dmask[p, k, j] = 1 if j >= p + 128*k  (valid, t <= s inside diag block)
    jj = np.arange(512)[None, None, :]
    pp = np.arange(P)[:, None, None]
    kk = np.arange(4)[None, :, None]
    dmask = (jj >= pp + P * kk).astype(NP_BF16)

    ident = np.eye(P, dtype=NP_BF16)
    ones_c = np.ones((P, 1), NP_BF16)
    ones_r = np.ones((1, P), np.float32)

    in_maps = []
    for c in range(N_CORES):
        qcols = np.concatenate(
            [(4 * c + h) * HEAD_DIM + perm for h in range(N_REP)])
        kcols = c * HEAD_DIM + perm
        vcols = c * HEAD_DIM + np.arange(HEAD_DIM)
        wq_c = np.ascontiguousarray(Wq[:, qcols]).astype(NP_BF16)
        wkv_c = np.ascontiguousarray(
            np.concatenate([Wk[:, kcols], Wv[:, vcols]], axis=1)).astype(NP_BF16)
        wo_c = np.ascontiguousarray(
            Wo[c * N_REP * HEAD_DIM:(c + 1) * N_REP * HEAD_DIM, :]).astype(NP_BF16)
        in_maps.append({
            "xT": xT, "wq": wq_c, "wkv": wkv_c, "wo": wo_c,
            "cos2": cos2, "sin2n": sin2n, "dmask": dmask,
            "ident": ident, "ones_c": ones_c, "ones_r": ones_r,
        })
    return in_maps


def get_nc():
    if "nc" not in _CACHED:
        _CACHED["nc"] = _build_nc()
    return _CACHED["nc"]


def kernel(x, start_pos, freqs_cos, freqs_sin, mask, cache_k, cache_v,
           Wq, Wk, Wv, Wo, _trace=False, _tmpdir=None):
    assert int(start_pos) == 0, "kernel hardcodes start_pos == 0"
    nc = get_nc()
    in_maps = _prep_inputs(x, freqs_cos, freqs_sin, Wq, Wk, Wv, Wo)
    kwargs = {}
    if _trace:
        kwargs = {"trace": True, "tmpdir": _tmpdir}
    res = run_bass_kernel_spmd(nc, in_maps, core_ids=list(range(N_CORES)), **kwargs)
    out = res.results[0]["y"].astype(np.float64)
    for c in range(1, N_CORES):
        out += res.results[c]["y"]
    out = out.astype(np.float32)
    if _trace:
        return out, res
    return out


# revision 12
# speedup vs baseline: 1.1421x; 1.1421x over previous
"""Trainium2 Bass kernel for nn_Attention_40407052320989.

Causal GQA attention block (Llama-style): QKV projection + RoPE + causal
softmax attention (8 KV heads, 32 Q heads, n_rep=4) + output projection.

Sharding: tensor-parallel over heads across 8 NeuronCores. Core c owns
KV head c and its 4 query heads: Wq/Wk/Wv column-sharded, Wo row-sharded
by the same head group. Each core computes a full [B, S, D] partial of
the output (its head group's contribution through Wo); the host sums the
8 partials (the row-parallel unshard).

On-chip layout: "feature on partitions, tokens on free dim" everywhere.
Scores are computed transposed (scoresT[t, s]) so the exp'd tile feeds
the PV matmul directly as the moving operand with no transposes. Softmax
sums over t (partition dim) ride a ones-vector matmul; the normalizer is
broadcast back over partitions with a rank-1 matmul and inverted as a
full-width DVE reciprocal.

Phase overlap: attention (phase 2) is Scalar-engine-bound (the exp
chain) while projections (phase 1) and the output projection (phase 3)
are TensorE-bound. Emission order interleaves P2(b0) with P1(b1) and
P2(b1) with P3(b0) so the scheduler overlaps them; PSUM is partitioned
4 banks for projections (two-pass: Q then KV, x re-streamed) and 4 for
attention, with phase 3 taking over projection banks afterwards.

RoPE pairing: Wq/Wk columns are permuted host-side so rotation pairs
(2i, 2i+1) land at partitions (i, 64+i) (rotate-half layout). Scores
are invariant to a consistent head-dim permutation of Wq and Wk. The
rotation is out = q * cos2 + swap_halves(q) * sin2n with cos2 = [cos;
cos] and sin2n = [-sin; sin]; the halves swap is two SBUF->SBUF DMAs.

Matmul operands are bf16 (measured: f32r runs ~2 cyc/row on HW, bf16 1
cyc/row); PSUM accumulation, softmax normalization, and the output stay
fp32. End-to-end numpy simulation of this precision mix: 4e-3 max rel.
"""

import math
import sys

sys.path.insert(0, "/opt/trn_rl_repo")

import ml_dtypes
import numpy as np

import concourse.bass as bass
import concourse.mybir as mybir
import concourse.tile as tile
from concourse import bacc
from concourse.bass_utils import run_bass_kernel_spmd

F32 = mybir.dt.float32
F32R = mybir.dt.float32r
BF16 = mybir.dt.bfloat16
AF = mybir.ActivationFunctionType
NP_BF16 = ml_dtypes.bfloat16

BSZ, SEQLEN, DIM = 2, 2048, 4096
N_HEADS, N_KV_HEADS, HEAD_DIM = 32, 8, 128
N_REP = N_HEADS // N_KV_HEADS  # q heads per core
N_CORES = 8
P = 128
NKD = DIM // P          # 32 contraction chunks for the projections
NT512 = SEQLEN // 512   # 4 blocks of 512 tokens per batch
NTC = SEQLEN // P       # 16 chunks of 128 tokens per batch
SCALE = 1.0 / math.sqrt(HEAD_DIM)

_CACHED = {}


def ts(i, n):
    return slice(i * n, (i + 1) * n)


class _Ctx:
    """Shared tiles/pools threaded through the phase emitters."""


def _build_nc():
    nc = bacc.Bacc(None, target_bir_lowering=False, debug=False)

    c = _Ctx()
    c.nc = nc
    c.xT = nc.declare_dram_parameter("xT", [BSZ, DIM, SEQLEN], BF16, isOutput=False)
    c.wq = nc.declare_dram_parameter("wq", [DIM, N_REP * HEAD_DIM], BF16, isOutput=False)
    c.wkv = nc.declare_dram_parameter("wkv", [DIM, 2 * HEAD_DIM], BF16, isOutput=False)
    c.wo = nc.declare_dram_parameter("wo", [N_REP * HEAD_DIM, DIM], BF16, isOutput=False)
    cos2 = nc.declare_dram_parameter("cos2", [P, SEQLEN], F32, isOutput=False)
    sin2n = nc.declare_dram_parameter("sin2n", [P, SEQLEN], F32, isOutput=False)
    dmask = nc.declare_dram_parameter("dmask", [P, 4, 512], BF16, isOutput=False)
    ident = nc.declare_dram_parameter("ident", [P, P], BF16, isOutput=False)
    ones_c = nc.declare_dram_parameter("ones_c", [P, 1], BF16, isOutput=False)
    ones_r = nc.declare_dram_parameter("ones_r", [1, P], F32R, isOutput=False)
    c.y = nc.declare_dram_parameter("y", [BSZ, SEQLEN, DIM], F32, isOutput=True)

    with tile.TileContext(nc) as tc, nc.allow_low_precision(
        reason="psum accumulation and normalization stay fp32 by construction"
    ):
        c.tc = tc
        with tc.tile_pool(name="const", bufs=1) as cpool:
            c.cos_sb = cpool.tile([P, SEQLEN], F32)
            c.sin_sb = cpool.tile([P, SEQLEN], F32)
            c.dm_sb = cpool.tile([P, 4, 512], BF16)
            c.id_sb = cpool.tile([P, P], BF16)
            c.onec_sb = cpool.tile([P, 1], BF16)
            c.oner_sb = cpool.tile([1, P], F32R)
            nc.sync.dma_start(c.cos_sb[:], cos2[:])
            nc.sync.dma_start(c.sin_sb[:], sin2n[:])
            nc.sync.dma_start(c.dm_sb[:], dmask[:])
            nc.sync.dma_start(c.id_sb[:], ident[:])
            nc.sync.dma_start(c.onec_sb[:], ones_c[:])
            nc.sync.dma_start(c.oner_sb[:], ones_r[:])
            _emit(c)

    nc.compile()
    return nc


def _emit(c):
    nc, tc = c.nc, c.tc
    with tc.tile_pool(name="big", bufs=1) as big, \
         tc.tile_pool(name="xs", bufs=6) as xpool, \
         tc.tile_pool(name="tp", bufs=2) as tpool, \
         tc.tile_pool(name="ep", bufs=6) as epool, \
         tc.tile_pool(name="np_", bufs=2) as npool:
        c.xpool, c.tpool, c.epool, c.npool = xpool, tpool, epool, npool

        c.wq_sb = big.tile([P, NKD, N_REP * HEAD_DIM], BF16)
        c.wkv_sb = big.tile([P, NKD, 2 * HEAD_DIM], BF16)
        c.wo_sb = big.tile([P, N_REP, DIM], BF16)
        wq_r = c.wq.rearrange("(o p) m -> p o m", p=P)
        wkv_r = c.wkv.rearrange("(o p) m -> p o m", p=P)
        for o4 in range(4):
            nc.sync.dma_start(c.wq_sb[:, ts(o4, 8), :], wq_r[:, ts(o4, 8), :])
            nc.sync.dma_start(c.wkv_sb[:, ts(o4, 8), :], wkv_r[:, ts(o4, 8), :])

        for b in range(2):
            with tc.tile_pool(name=f"qkv{b}", bufs=1) as qkv:
                c.qt = {b: qkv.tile([P, N_REP, SEQLEN], BF16, name=f"qt{b}")}
                c.kt = {b: qkv.tile([P, SEQLEN], BF16, name=f"kt{b}")}
                c.vn = {b: qkv.tile([P, NTC, HEAD_DIM], BF16, name=f"vn{b}")}
                c.ao = {b: qkv.tile([P, N_REP, SEQLEN], BF16, name=f"ao{b}")}
                with tc.tile_pool(name=f"p1_{b}", bufs=1, space="PSUM") as p1:
                    c.p1 = p1
                    for t5 in range(NT512):
                        _p1_block(c, b, t5)
                with tc.tile_pool(name=f"p2_{b}", bufs=1, space="PSUM") as p2:
                    c.p2 = p2
                    for h in range(N_REP):
                        _p2_head(c, b, h)
                if b == 0:
                    wo_r = c.wo.rearrange("(o p) n -> p o n", p=P)
                    for o4 in range(4):
                        nc.sync.dma_start(c.wo_sb[:, o4, :], wo_r[:, o4, :])
                with tc.tile_pool(name=f"p3_{b}", bufs=1, space="PSUM") as p3, \
                     tc.tile_pool(name=f"op{b}", bufs=3) as opool:
                    c.p3, c.opool = p3, opool
                    for s1 in range(NTC):
                        _p3_row(c, b, s1)


def _rope(c, out_slice, psum_in, tsl):
    """out = psum_in * cos2 + swap_halves(psum_in) * sin2n, [128, 512]."""
    nc = c.nc
    qf = c.tpool.tile([P, 512], F32, tag="rope_qf")
    rot = c.tpool.tile([P, 512], F32, tag="rope_rot")
    tmpa = c.tpool.tile([P, 512], F32, tag="rope_tmpa")
    nc.any.tensor_copy(qf[:], psum_in[:])
    nc.sync.dma_start(rot[0:64, :], qf[64:128, :])
    nc.sync.dma_start(rot[64:128, :], qf[0:64, :])
    nc.vector.tensor_mul(tmpa[:], qf[:], c.cos_sb[:, tsl])
    nc.vector.tensor_mul(rot[:], rot[:], c.sin_sb[:, tsl])
    nc.vector.tensor_add(out_slice, tmpa[:], rot[:])


def _p1_block(c, b, t5):
    """Projections for one 512-token block (single pass, 6 accumulators)."""
    nc = c.nc
    tsl = ts(t5, 512)
    pq = [c.p1.tile([P, 512], F32, tag=f"pq{h}", name=f"pq{h}")
          for h in range(N_REP)]
    pk = c.p1.tile([P, 512], F32, tag="pk")
    pv = c.p1.tile([P, 512], F32, tag="pv")
    for kd in range(NKD):
        xt = c.xpool.tile([P, 512], BF16, tag="xt")
        nc.sync.dma_start(xt[:], c.xT[b, ts(kd, P), tsl])
        st, sp = kd == 0, kd == NKD - 1
        for h in range(N_REP):
            nc.tensor.matmul(pq[h][:], c.wq_sb[:, kd, ts(h, P)], xt[:],
                             start=st, stop=sp)
        nc.tensor.matmul(pk[:], c.wkv_sb[:, kd, 0:P], xt[:], start=st, stop=sp)
        nc.tensor.matmul(pv[:], c.wkv_sb[:, kd, P:2 * P], xt[:], start=st, stop=sp)
    _rope(c, c.kt[b][:, tsl], pk, tsl)
    for h in range(N_REP):
        _rope(c, c.qt[b][:, h, tsl], pq[h], tsl)
    # V^T [d, t] -> V natural [t, d] via PE transpose
    vt_tmp = c.tpool.tile([P, 512], BF16, tag="vt_tmp")
    nc.any.tensor_copy(vt_tmp[:], pv[:])
    for j in range(4):
        pvt = c.p1.tile([P, P], BF16, tag="pvt", name="pvt")
        nc.tensor.transpose(pvt[:], vt_tmp[:, ts(j, P)], c.id_sb[:])
        nc.any.tensor_copy(c.vn[b][:, t5 * 4 + j, :], pvt[:])


def _p2_head(c, b, h):
    """Causal attention for one query head, scores transposed [t, s]."""
    nc = c.nc
    for s5 in range(NT512):
        po = c.p2.tile([P, 512], F32, tag="po", bufs=2)
        pz = c.p2.tile([1, 512], F32, tag="pz", bufs=2)
        ssl = ts(s5, 512)
        ntc = 4 * s5 + 4
        for tci in range(ntc):
            pscr = c.p2.tile([P, 512], F32, tag="ps", bufs=3, name="pscr")
            nc.tensor.matmul(pscr[:], c.kt[b][:, ts(tci, P)], c.qt[b][:, h, ssl],
                             start=True, stop=True)
            ex = c.epool.tile([P, 512], BF16, tag="ex")
            nc.scalar.activation(ex[:], pscr[:], AF.Exp, scale=SCALE)
            if tci >= 4 * s5:
                nc.vector.tensor_mul(ex[:], ex[:], c.dm_sb[:, tci - 4 * s5, :])
            st, sp = tci == 0, tci == ntc - 1
            nc.tensor.matmul(po[:], c.vn[b][:, tci, :], ex[:], start=st, stop=sp)
            nc.tensor.matmul(pz[:], c.onec_sb[:], ex[:], start=st, stop=sp)
        # broadcast sums over partitions, then invert at full width
        zs = c.npool.tile([1, 512], F32R, tag="zs")
        nc.vector.tensor_copy(zs[:], pz[:])
        pb = c.p2.tile([P, 512], F32, tag="pb", name="pb")
        nc.tensor.matmul(pb[:], c.oner_sb[:], zs[:], start=True, stop=True)
        rb = c.npool.tile([P, 512], F32, tag="rb")
        nc.vector.reciprocal(rb[:], pb[:])
        nc.vector.tensor_mul(c.ao[b][:, h, ssl], po[:], rb[:])


def _p3_row(c, b, s1):
    """Output projection for one 128-token row: two [128, 1024] groups x2."""
    nc = c.nc
    for half in range(4):
        pf = c.p3.tile([P, 2, 512], F32, tag="pf", bufs=2)
        for nq in range(2):
            n5 = half * 2 + nq
            for dh in range(N_REP):
                nc.tensor.matmul(pf[:, nq, :], c.ao[b][:, dh, ts(s1, P)],
                                 c.wo_sb[:, dh, ts(n5, 512)],
                                 start=dh == 0, stop=dh == N_REP - 1)
        ot = c.opool.tile([P, 1024], F32, tag="ot")
        nc.vector.tensor_copy(ot[:], pf[:])
        nc.sync.dma_start(c.y[b, ts(s1, P), ts(half, 1024)], ot[:])


def _prep_inputs(x, freqs_cos, freqs_sin, Wq, Wk, Wv, Wo):
    x = np.ascontiguousarray(np.asarray(x, dtype=np.float32))
    Wq = np.asarray(Wq, dtype=np.float32)
    Wk = np.asarray(Wk, dtype=np.float32)
    Wv = np.asarray(Wv, dtype=np.float32)
    Wo = np.asarray(Wo, dtype=np.float32)
    fc = np.asarray(freqs_cos, dtype=np.float32)
    fs = np.asarray(freqs_sin, dtype=np.float32)

    xT = np.ascontiguousarray(x.transpose(0, 2, 1)).astype(NP_BF16)  # [B, D, S]

    # rotate-half column permutation within each head
    perm = np.concatenate([np.arange(0, HEAD_DIM, 2), np.arange(1, HEAD_DIM, 2)])

    cos2 = np.concatenate([fc.T, fc.T], axis=0)       # [128, S]
    sin2n = np.concatenate([-fs.T, fs.T], axis=0)     # [128, S]

    # dmask[p, k, j] = 1 if j >= p + 128*k  (valid, t <= s inside diag block)
    jj = np.arange(512)[None, None, :]
    pp = np.arange(P)[:, None, None]
    kk = np.arange(4)[None, :, None]
    dmask = (jj >= pp + P * kk).astype(NP_BF16)

    ident = np.eye(P, dtype=NP_BF16)
    ones_c = np.ones((P, 1), NP_BF16)
    ones_r = np.ones((1, P), np.float32)

    in_maps = []
    for c in range(N_CORES):
        qcols = np.concatenate(
            [(4 * c + h) * HEAD_DIM + perm for h in range(N_REP)])
        kcols = c * HEAD_DIM + perm
        vcols = c * HEAD_DIM + np.arange(HEAD_DIM)
        wq_c = np.ascontiguousarray(Wq[:, qcols]).astype(NP_BF16)
        wkv_c = np.ascontiguousarray(
            np.concatenate([Wk[:, kcols], Wv[:, vcols]], axis=1)).astype(NP_BF16)
        wo_c = np.ascontiguousarray(
            Wo[c * N_REP * HEAD_DIM:(c + 1) * N_REP * HEAD_DIM, :]).astype(NP_BF16)
        in_maps.append({
            "xT": xT, "wq": wq_c, "wkv": wkv_c, "wo": wo_c,
            "cos2": cos2, "sin2n": sin2n, "dmask": dmask,
            "ident": ident, "ones_c": ones_c, "ones_r": ones_r,
        })
    return in_maps


def get_nc():
    if "nc" not in _CACHED:
        _CACHED["nc"] = _build_nc()
    return _CACHED["nc"]


def kernel(x, start_pos, freqs_cos, freqs_sin, mask, cache_k, cache_v,
           Wq, Wk, Wv, Wo, _trace=False, _tmpdir=None):
    assert int(start_pos) == 0, "kernel hardcodes start_pos == 0"
    nc = get_nc()
    in_maps = _prep_inputs(x, freqs_cos, freqs_sin, Wq, Wk, Wv, Wo)
    kwargs = {}
    if _trace:
        kwargs = {"trace": True, "tmpdir": _tmpdir}
    res = run_bass_kernel_spmd(nc, in_maps, core_ids=list(range(N_CORES)), **kwargs)
    out = res.results[0]["y"].astype(np.float64)
    for c in range(1, N_CORES):
        out += res.results[c]["y"]
    out = out.astype(np.float32)
    if _trace:
        return out, res
    return out


# revision 13
# speedup vs baseline: 1.1490x; 1.0060x over previous
"""Trainium2 Bass kernel for nn_Attention_40407052320989.

Causal GQA attention block (Llama-style): QKV projection + RoPE + causal
softmax attention (8 KV heads, 32 Q heads, n_rep=4) + output projection.

Sharding: tensor-parallel over heads across 8 NeuronCores. Core c owns
KV head c and its 4 query heads: Wq/Wk/Wv column-sharded, Wo row-sharded
by the same head group. Each core computes a full [B, S, D] partial of
the output (its head group's contribution through Wo); the host sums the
8 partials (the row-parallel unshard).

On-chip layout: "feature on partitions, tokens on free dim" everywhere.
Scores are computed transposed (scoresT[t, s]) so the exp'd tile feeds
the PV matmul directly as the moving operand with no transposes. Softmax
sums over t (partition dim) ride a ones-vector matmul; the normalizer is
broadcast back over partitions with a rank-1 matmul and inverted as a
full-width DVE reciprocal.

Phase overlap: attention (phase 2) is Scalar-engine-bound (the exp
chain) while projections (phase 1) and the output projection (phase 3)
are TensorE-bound. Emission order interleaves P2(b0) with P1(b1) and
P2(b1) with P3(b0) so the scheduler overlaps them; PSUM is partitioned
4 banks for projections (two-pass: Q then KV, x re-streamed) and 4 for
attention, with phase 3 taking over projection banks afterwards.

RoPE pairing: Wq/Wk columns are permuted host-side so rotation pairs
(2i, 2i+1) land at partitions (i, 64+i) (rotate-half layout). Scores
are invariant to a consistent head-dim permutation of Wq and Wk. The
rotation is out = q * cos2 + swap_halves(q) * sin2n with cos2 = [cos;
cos] and sin2n = [-sin; sin]; the halves swap is two SBUF->SBUF DMAs.

Matmul operands are bf16 (measured: f32r runs ~2 cyc/row on HW, bf16 1
cyc/row); PSUM accumulation, softmax normalization, and the output stay
fp32. End-to-end numpy simulation of this precision mix: 4e-3 max rel.
"""

import math
import sys

sys.path.insert(0, "/opt/trn_rl_repo")

import ml_dtypes
import numpy as np

import concourse.bass as bass
import concourse.mybir as mybir
import concourse.tile as tile
from concourse import bacc
from concourse.bass_utils import run_bass_kernel_spmd

F32 = mybir.dt.float32
F32R = mybir.dt.float32r
BF16 = mybir.dt.bfloat16
AF = mybir.ActivationFunctionType
NP_BF16 = ml_dtypes.bfloat16

BSZ, SEQLEN, DIM = 2, 2048, 4096
N_HEADS, N_KV_HEADS, HEAD_DIM = 32, 8, 128
N_REP = N_HEADS // N_KV_HEADS  # q heads per core
N_CORES = 8
P = 128
NKD = DIM // P          # 32 contraction chunks for the projections
NT512 = SEQLEN // 512   # 4 blocks of 512 tokens per batch
NTC = SEQLEN // P       # 16 chunks of 128 tokens per batch
SCALE = 1.0 / math.sqrt(HEAD_DIM)

_CACHED = {}


def ts(i, n):
    return slice(i * n, (i + 1) * n)


class _Ctx:
    """Shared tiles/pools threaded through the phase emitters."""


def _build_nc():
    nc = bacc.Bacc(None, target_bir_lowering=False, debug=False)

    c = _Ctx()
    c.nc = nc
    c.xT = nc.declare_dram_parameter("xT", [BSZ, DIM, SEQLEN], BF16, isOutput=False)
    c.wq = nc.declare_dram_parameter("wq", [DIM, N_REP * HEAD_DIM], BF16, isOutput=False)
    c.wkv = nc.declare_dram_parameter("wkv", [DIM, 2 * HEAD_DIM], BF16, isOutput=False)
    c.wo = nc.declare_dram_parameter("wo", [N_REP * HEAD_DIM, DIM], BF16, isOutput=False)
    cos2 = nc.declare_dram_parameter("cos2", [P, SEQLEN], F32, isOutput=False)
    sin2n = nc.declare_dram_parameter("sin2n", [P, SEQLEN], F32, isOutput=False)
    dmask = nc.declare_dram_parameter("dmask", [P, 4, 512], BF16, isOutput=False)
    ident = nc.declare_dram_parameter("ident", [P, P], BF16, isOutput=False)
    ones_c = nc.declare_dram_parameter("ones_c", [P, 1], BF16, isOutput=False)
    ones_r = nc.declare_dram_parameter("ones_r", [1, P], F32R, isOutput=False)
    c.y = nc.declare_dram_parameter("y", [BSZ, SEQLEN, DIM], F32, isOutput=True)

    with tile.TileContext(nc) as tc, nc.allow_low_precision(
        reason="psum accumulation and normalization stay fp32 by construction"
    ):
        c.tc = tc
        with tc.tile_pool(name="const", bufs=1) as cpool:
            c.cos_sb = cpool.tile([P, SEQLEN], F32)
            c.sin_sb = cpool.tile([P, SEQLEN], F32)
            c.dm_sb = cpool.tile([P, 4, 512], BF16)
            c.id_sb = cpool.tile([P, P], BF16)
            c.onec_sb = cpool.tile([P, 1], BF16)
            c.oner_sb = cpool.tile([1, P], F32R)
            nc.sync.dma_start(c.cos_sb[:], cos2[:])
            nc.sync.dma_start(c.sin_sb[:], sin2n[:])
            nc.sync.dma_start(c.dm_sb[:], dmask[:])
            nc.sync.dma_start(c.id_sb[:], ident[:])
            nc.sync.dma_start(c.onec_sb[:], ones_c[:])
            nc.sync.dma_start(c.oner_sb[:], ones_r[:])
            _emit(c)

    nc.compile()
    return nc


def _emit(c):
    nc, tc = c.nc, c.tc
    with tc.tile_pool(name="big", bufs=1) as big, \
         tc.tile_pool(name="xs", bufs=6) as xpool, \
         tc.tile_pool(name="tp", bufs=2) as tpool, \
         tc.tile_pool(name="ep", bufs=6) as epool, \
         tc.tile_pool(name="np_", bufs=2) as npool:
        c.xpool, c.tpool, c.epool, c.npool = xpool, tpool, epool, npool

        c.wq_sb = big.tile([P, NKD, N_REP * HEAD_DIM], BF16)
        c.wkv_sb = big.tile([P, NKD, 2 * HEAD_DIM], BF16)
        c.wo_sb = big.tile([P, N_REP, DIM], BF16)
        wq_r = c.wq.rearrange("(o p) m -> p o m", p=P)
        wkv_r = c.wkv.rearrange("(o p) m -> p o m", p=P)
        for o4 in range(4):
            nc.sync.dma_start(c.wq_sb[:, ts(o4, 8), :], wq_r[:, ts(o4, 8), :])
            nc.sync.dma_start(c.wkv_sb[:, ts(o4, 8), :], wkv_r[:, ts(o4, 8), :])

        for b in range(2):
            with tc.tile_pool(name=f"qkv{b}", bufs=1) as qkv:
                c.qt = {b: qkv.tile([P, N_REP, SEQLEN], BF16, name=f"qt{b}")}
                c.kt = {b: qkv.tile([P, SEQLEN], BF16, name=f"kt{b}")}
                c.vn = {b: qkv.tile([P, NTC, HEAD_DIM], BF16, name=f"vn{b}")}
                c.ao = {b: qkv.tile([P, N_REP, SEQLEN], BF16, name=f"ao{b}")}
                with tc.tile_pool(name=f"p1_{b}", bufs=1, space="PSUM") as p1:
                    c.p1 = p1
                    for t5 in range(NT512):
                        _p1_block(c, b, t5)
                with tc.tile_pool(name=f"p2_{b}", bufs=1, space="PSUM") as p2:
                    c.p2 = p2
                    for h in range(N_REP):
                        _p2_head(c, b, h)
                if b == 0:
                    wo_r = c.wo.rearrange("(o p) n -> p o n", p=P)
                    for o4 in range(4):
                        nc.sync.dma_start(c.wo_sb[:, o4, :], wo_r[:, o4, :])
                with tc.tile_pool(name=f"p3_{b}", bufs=1, space="PSUM") as p3, \
                     tc.tile_pool(name=f"op{b}", bufs=3) as opool:
                    c.p3, c.opool = p3, opool
                    for s1 in range(NTC):
                        _p3_row(c, b, s1)


def _rope(c, out_slice, psum_in, tsl):
    """out = psum_in * cos2 + swap_halves(psum_in) * sin2n, [128, 512]."""
    nc = c.nc
    qf = c.tpool.tile([P, 512], F32, tag="rope_qf")
    rot = c.tpool.tile([P, 512], F32, tag="rope_rot")
    tmpa = c.tpool.tile([P, 512], F32, tag="rope_tmpa")
    nc.any.tensor_copy(qf[:], psum_in[:])
    nc.sync.dma_start(rot[0:64, :], qf[64:128, :])
    nc.sync.dma_start(rot[64:128, :], qf[0:64, :])
    nc.vector.tensor_mul(tmpa[:], qf[:], c.cos_sb[:, tsl])
    nc.vector.tensor_mul(rot[:], rot[:], c.sin_sb[:, tsl])
    nc.vector.tensor_add(out_slice, tmpa[:], rot[:])


def _p1_block(c, b, t5):
    """Projections for one 512-token block (single pass, 6 accumulators)."""
    nc = c.nc
    tsl = ts(t5, 512)
    pq = [c.p1.tile([P, 512], F32, tag=f"pq{h}", name=f"pq{h}")
          for h in range(N_REP)]
    pk = c.p1.tile([P, 512], F32, tag="pk")
    pv = c.p1.tile([P, 512], F32, tag="pv")
    for kd in range(NKD):
        xt = c.xpool.tile([P, 512], BF16, tag="xt")
        nc.sync.dma_start(xt[:], c.xT[b, ts(kd, P), tsl])
        st, sp = kd == 0, kd == NKD - 1
        for h in range(N_REP):
            nc.tensor.matmul(pq[h][:], c.wq_sb[:, kd, ts(h, P)], xt[:],
                             start=st, stop=sp)
        nc.tensor.matmul(pk[:], c.wkv_sb[:, kd, 0:P], xt[:], start=st, stop=sp)
        nc.tensor.matmul(pv[:], c.wkv_sb[:, kd, P:2 * P], xt[:], start=st, stop=sp)
    _rope(c, c.kt[b][:, tsl], pk, tsl)
    for h in range(N_REP):
        _rope(c, c.qt[b][:, h, tsl], pq[h], tsl)
    # V^T [d, t] -> V natural [t, d] via PE transpose
    vt_tmp = c.tpool.tile([P, 512], BF16, tag="vt_tmp")
    nc.any.tensor_copy(vt_tmp[:], pv[:])
    for j in range(4):
        pvt = c.p1.tile([P, P], BF16, tag="pvt", name="pvt")
        nc.tensor.transpose(pvt[:], vt_tmp[:, ts(j, P)], c.id_sb[:])
        nc.any.tensor_copy(c.vn[b][:, t5 * 4 + j, :], pvt[:])


def _p2_head(c, b, h):
    """Causal attention for one query head, scores transposed [t, s]."""
    nc = c.nc
    for s5 in range(NT512):
        po = c.p2.tile([P, 512], F32, tag="po", bufs=2)
        pz = c.p2.tile([1, 512], F32, tag="pzb", bufs=2, name="pz")
        ssl = ts(s5, 512)
        ntc = 4 * s5 + 4
        for tci in range(ntc):
            pscr = c.p2.tile([P, 512], F32, tag="ps", bufs=4, name="pscr")
            nc.tensor.matmul(pscr[:], c.kt[b][:, ts(tci, P)], c.qt[b][:, h, ssl],
                             start=True, stop=True)
            ex = c.epool.tile([P, 512], BF16, tag="ex")
            nc.scalar.activation(ex[:], pscr[:], AF.Exp, scale=SCALE)
            if tci >= 4 * s5:
                nc.vector.tensor_mul(ex[:], ex[:], c.dm_sb[:, tci - 4 * s5, :])
            st, sp = tci == 0, tci == ntc - 1
            nc.tensor.matmul(po[:], c.vn[b][:, tci, :], ex[:], start=st, stop=sp)
            nc.tensor.matmul(pz[:], c.onec_sb[:], ex[:], start=st, stop=sp)
        # broadcast sums over partitions, then invert at full width
        zs = c.npool.tile([1, 512], F32R, tag="zs")
        nc.vector.tensor_copy(zs[:], pz[:])
        pb = c.p2.tile([P, 512], F32, tag="pzb", bufs=2, name="pb")
        nc.tensor.matmul(pb[:], c.oner_sb[:], zs[:], start=True, stop=True)
        rb = c.npool.tile([P, 512], F32, tag="rb")
        nc.vector.reciprocal(rb[:], pb[:])
        nc.vector.tensor_mul(c.ao[b][:, h, ssl], po[:], rb[:])


def _p3_row(c, b, s1):
    """Output projection for one 128-token row: two [128, 1024] groups x2."""
    nc = c.nc
    for half in range(4):
        pf = c.p3.tile([P, 2, 512], F32, tag="pf", bufs=2)
        for nq in range(2):
            n5 = half * 2 + nq
            for dh in range(N_REP):
                nc.tensor.matmul(pf[:, nq, :], c.ao[b][:, dh, ts(s1, P)],
                                 c.wo_sb[:, dh, ts(n5, 512)],
                                 start=dh == 0, stop=dh == N_REP - 1)
        ot = c.opool.tile([P, 1024], F32, tag="ot")
        nc.vector.tensor_copy(ot[:], pf[:])
        nc.sync.dma_start(c.y[b, ts(s1, P), ts(half, 1024)], ot[:])


def _prep_inputs(x, freqs_cos, freqs_sin, Wq, Wk, Wv, Wo):
    x = np.ascontiguousarray(np.asarray(x, dtype=np.float32))
    Wq = np.asarray(Wq, dtype=np.float32)
    Wk = np.asarray(Wk, dtype=np.float32)
    Wv = np.asarray(Wv, dtype=np.float32)
    Wo = np.asarray(Wo, dtype=np.float32)
    fc = np.asarray(freqs_cos, dtype=np.float32)
    fs = np.asarray(freqs_sin, dtype=np.float32)

    xT = np.ascontiguousarray(x.transpose(0, 2, 1)).astype(NP_BF16)  # [B, D, S]

    # rotate-half column permutation within each head
    perm = np.concatenate([np.arange(0, HEAD_DIM, 2), np.arange(1, HEAD_DIM, 2)])

    cos2 = np.concatenate([fc.T, fc.T], axis=0)       # [128, S]
    sin2n = np.concatenate([-fs.T, fs.T], axis=0)     # [128, S]

    # dmask[p, k, j] = 1 if j >= p + 128*k  (valid, t <= s inside diag block)
    jj = np.arange(512)[None, None, :]
    pp = np.arange(P)[:, None, None]
    kk = np.arange(4)[None, :, None]
    dmask = (jj >= pp + P * kk).astype(NP_BF16)

    ident = np.eye(P, dtype=NP_BF16)
    ones_c = np.ones((P, 1), NP_BF16)
    ones_r = np.ones((1, P), np.float32)

    in_maps = []
    for c in range(N_CORES):
        qcols = np.concatenate(
            [(4 * c + h) * HEAD_DIM + perm for h in range(N_REP)])
        kcols = c * HEAD_DIM + perm
        vcols = c * HEAD_DIM + np.arange(HEAD_DIM)
        wq_c = np.ascontiguousarray(Wq[:, qcols]).astype(NP_BF16)
        wkv_c = np.ascontiguousarray(
            np.concatenate([Wk[:, kcols], Wv[:, vcols]], axis=1)).astype(NP_BF16)
        wo_c = np.ascontiguousarray(
            Wo[c * N_REP * HEAD_DIM:(c + 1) * N_REP * HEAD_DIM, :]).astype(NP_BF16)
        in_maps.append({
            "xT": xT, "wq": wq_c, "wkv": wkv_c, "wo": wo_c,
            "cos2": cos2, "sin2n": sin2n, "dmask": dmask,
            "ident": ident, "ones_c": ones_c, "ones_r": ones_r,
        })
    return in_maps


def get_nc():
    if "nc" not in _CACHED:
        _CACHED["nc"] = _build_nc()
    return _CACHED["nc"]


def kernel(x, start_pos, freqs_cos, freqs_sin, mask, cache_k, cache_v,
           Wq, Wk, Wv, Wo, _trace=False, _tmpdir=None):
    assert int(start_pos) == 0, "kernel hardcodes start_pos == 0"
    nc = get_nc()
    in_maps = _prep_inputs(x, freqs_cos, freqs_sin, Wq, Wk, Wv, Wo)
    kwargs = {}
    if _trace:
        kwargs = {"trace": True, "tmpdir": _tmpdir}
    res = run_bass_kernel_spmd(nc, in_maps, core_ids=list(range(N_CORES)), **kwargs)
    out = res.results[0]["y"].astype(np.float64)
    for c in range(1, N_CORES):
        out += res.results[c]["y"]
    out = out.astype(np.float32)
    if _trace:
        return out, res
    return out


# revision 14
# speedup vs baseline: 1.1778x; 1.0251x over previous
"""Trainium2 Bass kernel for nn_Attention_40407052320989.

Causal GQA attention block (Llama-style): QKV projection + RoPE + causal
softmax attention (8 KV heads, 32 Q heads, n_rep=4) + output projection.

Sharding: tensor-parallel over heads across 8 NeuronCores. Core c owns
KV head c and its 4 query heads: Wq/Wk/Wv column-sharded, Wo row-sharded
by the same head group. Each core computes a full [B, S, D] partial of
the output (its head group's contribution through Wo); the host sums the
8 partials (the row-parallel unshard).

On-chip layout: "feature on partitions, tokens on free dim" everywhere.
Scores are computed transposed (scoresT[t, s]) so the exp'd tile feeds
the PV matmul directly as the moving operand with no transposes. Softmax
sums over t (partition dim) ride a ones-vector matmul; the normalizer is
broadcast back over partitions with a rank-1 matmul and inverted as a
full-width DVE reciprocal.

Phase overlap: attention (phase 2) is Scalar-engine-bound (the exp
chain) while projections (phase 1) and the output projection (phase 3)
are TensorE-bound. Emission order interleaves P2(b0) with P1(b1) and
P2(b1) with P3(b0) so the scheduler overlaps them; PSUM is partitioned
4 banks for projections (two-pass: Q then KV, x re-streamed) and 4 for
attention, with phase 3 taking over projection banks afterwards.

RoPE pairing: Wq/Wk columns are permuted host-side so rotation pairs
(2i, 2i+1) land at partitions (i, 64+i) (rotate-half layout). Scores
are invariant to a consistent head-dim permutation of Wq and Wk. The
rotation is out = q * cos2 + swap_halves(q) * sin2n with cos2 = [cos;
cos] and sin2n = [-sin; sin]; the halves swap is two SBUF->SBUF DMAs.

Matmul operands are bf16 (measured: f32r runs ~2 cyc/row on HW, bf16 1
cyc/row); PSUM accumulation, softmax normalization, and the output stay
fp32. End-to-end numpy simulation of this precision mix: 4e-3 max rel.
"""

import math
import sys

sys.path.insert(0, "/opt/trn_rl_repo")

import ml_dtypes
import numpy as np

import concourse.bass as bass
import concourse.mybir as mybir
import concourse.tile as tile
from concourse import bacc
from concourse.bass_utils import run_bass_kernel_spmd

F32 = mybir.dt.float32
F32R = mybir.dt.float32r
BF16 = mybir.dt.bfloat16
AF = mybir.ActivationFunctionType
NP_BF16 = ml_dtypes.bfloat16

BSZ, SEQLEN, DIM = 2, 2048, 4096
N_HEADS, N_KV_HEADS, HEAD_DIM = 32, 8, 128
N_REP = N_HEADS // N_KV_HEADS  # q heads per core
N_CORES = 8
P = 128
NKD = DIM // P          # 32 contraction chunks for the projections
NT512 = SEQLEN // 512   # 4 blocks of 512 tokens per batch
NTC = SEQLEN // P       # 16 chunks of 128 tokens per batch
SCALE = 1.0 / math.sqrt(HEAD_DIM)

_CACHED = {}


def ts(i, n):
    return slice(i * n, (i + 1) * n)


class _Ctx:
    """Shared tiles/pools threaded through the phase emitters."""


def _build_nc():
    nc = bacc.Bacc(None, target_bir_lowering=False, debug=False)

    c = _Ctx()
    c.nc = nc
    c.xT = nc.declare_dram_parameter("xT", [BSZ, DIM, SEQLEN], BF16, isOutput=False)
    c.wq = nc.declare_dram_parameter("wq", [DIM, N_REP * HEAD_DIM], BF16, isOutput=False)
    c.wkv = nc.declare_dram_parameter("wkv", [DIM, 2 * HEAD_DIM], BF16, isOutput=False)
    c.wo = nc.declare_dram_parameter("wo", [N_REP * HEAD_DIM, DIM], BF16, isOutput=False)
    cos2 = nc.declare_dram_parameter("cos2", [P, SEQLEN], F32, isOutput=False)
    sin2n = nc.declare_dram_parameter("sin2n", [P, SEQLEN], F32, isOutput=False)
    dmask = nc.declare_dram_parameter("dmask", [P, 4, 512], BF16, isOutput=False)
    ident = nc.declare_dram_parameter("ident", [P, P], BF16, isOutput=False)
    ones_c = nc.declare_dram_parameter("ones_c", [P, 1], BF16, isOutput=False)
    ones_r = nc.declare_dram_parameter("ones_r", [1, P], F32R, isOutput=False)
    c.y = nc.declare_dram_parameter("y", [BSZ, SEQLEN, DIM], F32, isOutput=True)

    with tile.TileContext(nc) as tc, nc.allow_low_precision(
        reason="psum accumulation and normalization stay fp32 by construction"
    ):
        c.tc = tc
        with tc.tile_pool(name="const", bufs=1) as cpool:
            c.cos_sb = cpool.tile([P, SEQLEN], F32)
            c.sin_sb = cpool.tile([P, SEQLEN], F32)
            c.dm_sb = cpool.tile([P, 4, 512], BF16)
            c.id_sb = cpool.tile([P, P], BF16)
            c.onec_sb = cpool.tile([P, 1], BF16)
            c.oner_sb = cpool.tile([1, P], F32R)
            nc.sync.dma_start(c.cos_sb[:], cos2[:])
            nc.sync.dma_start(c.sin_sb[:], sin2n[:])
            nc.sync.dma_start(c.dm_sb[:], dmask[:])
            nc.sync.dma_start(c.id_sb[:], ident[:])
            nc.sync.dma_start(c.onec_sb[:], ones_c[:])
            nc.sync.dma_start(c.oner_sb[:], ones_r[:])
            _emit(c)

    nc.compile()
    return nc


def _emit(c):
    nc, tc = c.nc, c.tc
    with tc.tile_pool(name="big", bufs=1) as big, \
         tc.tile_pool(name="xs", bufs=8) as xpool, \
         tc.tile_pool(name="tp", bufs=2) as tpool, \
         tc.tile_pool(name="ep", bufs=6) as epool, \
         tc.tile_pool(name="np_", bufs=2) as npool:
        c.xpool, c.tpool, c.epool, c.npool = xpool, tpool, epool, npool

        c.wq_sb = big.tile([P, NKD, N_REP * HEAD_DIM], BF16)
        c.wkv_sb = big.tile([P, NKD, 2 * HEAD_DIM], BF16)
        c.wo_sb = big.tile([P, N_REP, DIM], BF16)
        wq_r = c.wq.rearrange("(o p) m -> p o m", p=P)
        wkv_r = c.wkv.rearrange("(o p) m -> p o m", p=P)
        for o4 in range(4):
            nc.sync.dma_start(c.wq_sb[:, ts(o4, 8), :], wq_r[:, ts(o4, 8), :])
            nc.sync.dma_start(c.wkv_sb[:, ts(o4, 8), :], wkv_r[:, ts(o4, 8), :])

        for b in range(2):
            with tc.tile_pool(name=f"qkv{b}", bufs=1) as qkv:
                c.qt = {b: qkv.tile([P, N_REP, SEQLEN], BF16, name=f"qt{b}")}
                c.kt = {b: qkv.tile([P, SEQLEN], BF16, name=f"kt{b}")}
                c.vn = {b: qkv.tile([P, NTC, HEAD_DIM], BF16, name=f"vn{b}")}
                c.ao = {b: qkv.tile([P, N_REP, SEQLEN], BF16, name=f"ao{b}")}
                with tc.tile_pool(name=f"p1_{b}", bufs=1, space="PSUM") as p1:
                    c.p1 = p1
                    for t5 in range(NT512):
                        _p1_block(c, b, t5)
                with tc.tile_pool(name=f"p2_{b}", bufs=1, space="PSUM") as p2:
                    c.p2 = p2
                    for h in range(N_REP):
                        _p2_head(c, b, h)
                if b == 0:
                    wo_r = c.wo.rearrange("(o p) n -> p o n", p=P)
                    for o4 in range(4):
                        nc.sync.dma_start(c.wo_sb[:, o4, :], wo_r[:, o4, :])
                with tc.tile_pool(name=f"p3_{b}", bufs=1, space="PSUM") as p3, \
                     tc.tile_pool(name=f"op{b}", bufs=3) as opool:
                    c.p3, c.opool = p3, opool
                    for s1 in range(NTC):
                        _p3_row(c, b, s1)


def _rope(c, out_slice, psum_in, tsl):
    """out = psum_in * cos2 + swap_halves(psum_in) * sin2n, [128, 512]."""
    nc = c.nc
    qf = c.tpool.tile([P, 512], F32, tag="rope_qf")
    rot = c.tpool.tile([P, 512], F32, tag="rope_rot")
    tmpa = c.tpool.tile([P, 512], F32, tag="rope_tmpa")
    nc.any.tensor_copy(qf[:], psum_in[:])
    nc.sync.dma_start(rot[0:64, :], qf[64:128, :])
    nc.sync.dma_start(rot[64:128, :], qf[0:64, :])
    nc.vector.tensor_mul(tmpa[:], qf[:], c.cos_sb[:, tsl])
    nc.vector.tensor_mul(rot[:], rot[:], c.sin_sb[:, tsl])
    nc.vector.tensor_add(out_slice, tmpa[:], rot[:])


def _p1_block(c, b, t5):
    """Projections for one 512-token block (single pass, 6 accumulators)."""
    nc = c.nc
    tsl = ts(t5, 512)
    pq = [c.p1.tile([P, 512], F32, tag=f"pq{h}", name=f"pq{h}")
          for h in range(N_REP)]
    pk = c.p1.tile([P, 512], F32, tag="pk")
    pv = c.p1.tile([P, 512], F32, tag="pv")
    for kd in range(NKD):
        xt = c.xpool.tile([P, 512], BF16, tag="xt")
        nc.sync.dma_start(xt[:], c.xT[b, ts(kd, P), tsl])
        st, sp = kd == 0, kd == NKD - 1
        for h in range(N_REP):
            nc.tensor.matmul(pq[h][:], c.wq_sb[:, kd, ts(h, P)], xt[:],
                             start=st, stop=sp)
        nc.tensor.matmul(pk[:], c.wkv_sb[:, kd, 0:P], xt[:], start=st, stop=sp)
        nc.tensor.matmul(pv[:], c.wkv_sb[:, kd, P:2 * P], xt[:], start=st, stop=sp)
    _rope(c, c.kt[b][:, tsl], pk, tsl)
    for h in range(N_REP):
        _rope(c, c.qt[b][:, h, tsl], pq[h], tsl)
    # V^T [d, t] -> V natural [t, d] via PE transpose
    vt_tmp = c.tpool.tile([P, 512], BF16, tag="vt_tmp")
    nc.any.tensor_copy(vt_tmp[:], pv[:])
    for j in range(4):
        pvt = c.p1.tile([P, P], BF16, tag="pvt", name="pvt")
        nc.tensor.transpose(pvt[:], vt_tmp[:, ts(j, P)], c.id_sb[:])
        nc.any.tensor_copy(c.vn[b][:, t5 * 4 + j, :], pvt[:])


def _p2_head(c, b, h):
    """Causal attention for one query head, scores transposed [t, s]."""
    nc = c.nc
    for s5 in range(NT512):
        po = c.p2.tile([P, 512], F32, tag="po", bufs=2)
        pz = c.p2.tile([1, 512], F32, tag="pzb", bufs=2, name="pz")
        ssl = ts(s5, 512)
        ntc = 4 * s5 + 4
        for tci in range(ntc):
            pscr = c.p2.tile([P, 512], F32, tag="ps", bufs=4, name="pscr")
            nc.tensor.matmul(pscr[:], c.kt[b][:, ts(tci, P)], c.qt[b][:, h, ssl],
                             start=True, stop=True)
            ex = c.epool.tile([P, 512], BF16, tag="ex")
            nc.scalar.activation(ex[:], pscr[:], AF.Exp, scale=SCALE)
            if tci >= 4 * s5:
                nc.gpsimd.tensor_mul(ex[:], ex[:], c.dm_sb[:, tci - 4 * s5, :])
            st, sp = tci == 0, tci == ntc - 1
            nc.tensor.matmul(po[:], c.vn[b][:, tci, :], ex[:], start=st, stop=sp)
            nc.tensor.matmul(pz[:], c.onec_sb[:], ex[:], start=st, stop=sp)
        # broadcast sums over partitions, then invert at full width
        zs = c.npool.tile([1, 512], F32R, tag="zs")
        nc.vector.tensor_copy(zs[:], pz[:])
        pb = c.p2.tile([P, 512], F32, tag="pzb", bufs=2, name="pb")
        nc.tensor.matmul(pb[:], c.oner_sb[:], zs[:], start=True, stop=True)
        rb = c.npool.tile([P, 512], F32, tag="rb")
        nc.vector.reciprocal(rb[:], pb[:])
        nc.vector.tensor_mul(c.ao[b][:, h, ssl], po[:], rb[:])


def _p3_row(c, b, s1):
    """Output projection for one 128-token row: two [128, 1024] groups x2."""
    nc = c.nc
    for half in range(4):
        pf = c.p3.tile([P, 2, 512], F32, tag="pf", bufs=2)
        for nq in range(2):
            n5 = half * 2 + nq
            for dh in range(N_REP):
                nc.tensor.matmul(pf[:, nq, :], c.ao[b][:, dh, ts(s1, P)],
                                 c.wo_sb[:, dh, ts(n5, 512)],
                                 start=dh == 0, stop=dh == N_REP - 1)
        ot = c.opool.tile([P, 1024], F32, tag="ot")
        nc.vector.tensor_copy(ot[:], pf[:])
        nc.sync.dma_start(c.y[b, ts(s1, P), ts(half, 1024)], ot[:])


def _prep_inputs(x, freqs_cos, freqs_sin, Wq, Wk, Wv, Wo):
    x = np.ascontiguousarray(np.asarray(x, dtype=np.float32))
    Wq = np.asarray(Wq, dtype=np.float32)
    Wk = np.asarray(Wk, dtype=np.float32)
    Wv = np.asarray(Wv, dtype=np.float32)
    Wo = np.asarray(Wo, dtype=np.float32)
    fc = np.asarray(freqs_cos, dtype=np.float32)
    fs = np.asarray(freqs_sin, dtype=np.float32)

    xT = np.ascontiguousarray(x.transpose(0, 2, 1)).astype(NP_BF16)  # [B, D, S]

    # rotate-half column permutation within each head
    perm = np.concatenate([np.arange(0, HEAD_DIM, 2), np.arange(1, HEAD_DIM, 2)])

    cos2 = np.concatenate([fc.T, fc.T], axis=0)       # [128, S]
    sin2n = np.concatenate([-fs.T, fs.T], axis=0)     # [128, S]

    # dmask[p, k, j] = 1 if j >= p + 128*k  (valid, t <= s inside diag block)
    jj = np.arange(512)[None, None, :]
    pp = np.arange(P)[:, None, None]
    kk = np.arange(4)[None, :, None]
    dmask = (jj >= pp + P * kk).astype(NP_BF16)

    ident = np.eye(P, dtype=NP_BF16)
    ones_c = np.ones((P, 1), NP_BF16)
    ones_r = np.ones((1, P), np.float32)

    in_maps = []
    for c in range(N_CORES):
        qcols = np.concatenate(
            [(4 * c + h) * HEAD_DIM + perm for h in range(N_REP)])
        kcols = c * HEAD_DIM + perm
        vcols = c * HEAD_DIM + np.arange(HEAD_DIM)
        wq_c = np.ascontiguousarray(Wq[:, qcols]).astype(NP_BF16)
        wkv_c = np.ascontiguousarray(
            np.concatenate([Wk[:, kcols], Wv[:, vcols]], axis=1)).astype(NP_BF16)
        wo_c = np.ascontiguousarray(
            Wo[c * N_REP * HEAD_DIM:(c + 1) * N_REP * HEAD_DIM, :]).astype(NP_BF16)
        in_maps.append({
            "xT": xT, "wq": wq_c, "wkv": wkv_c, "wo": wo_c,
            "cos2": cos2, "sin2n": sin2n, "dmask": dmask,
            "ident": ident, "ones_c": ones_c, "ones_r": ones_r,
        })
    return in_maps


def get_nc():
    if "nc" not in _CACHED:
        _CACHED["nc"] = _build_nc()
    return _CACHED["nc"]


def kernel(x, start_pos, freqs_cos, freqs_sin, mask, cache_k, cache_v,
           Wq, Wk, Wv, Wo, _trace=False, _tmpdir=None):
    assert int(start_pos) == 0, "kernel hardcodes start_pos == 0"
    nc = get_nc()
    in_maps = _prep_inputs(x, freqs_cos, freqs_sin, Wq, Wk, Wv, Wo)
    kwargs = {}
    if _trace:
        kwargs = {"trace": True, "tmpdir": _tmpdir}
    res = run_bass_kernel_spmd(nc, in_maps, core_ids=list(range(N_CORES)), **kwargs)
    out = res.results[0]["y"].astype(np.float64)
    for c in range(1, N_CORES):
        out += res.results[c]["y"]
    out = out.astype(np.float32)
    if _trace:
        return out, res
    return out
